# revision 1
# baseline (speedup 1.0000x reference)
"""DMN encoder (3-hop masked-attention message passing) on 8 trn2 cores.

Sharding: pure data-parallel over the batch dim (16 rows/core). Each core:
  - loads its value slice as bf16 (SWDGE cast during DMA),
  - computes hop-invariant per-neighbor dots vs = V.wf, vu = V.wu via
    PE-transpose + matmul,
  - runs the 3-hop recurrence where softmax numerators are
    num_h = mask * max(exp(vs) * exp(c_h), 1)   (exact relu-softmax identity),
    and hop-1's scalar c_1 needs only sum(num0*vu)/D0 (no [B,N,D] pass),
  - streams V twice through the PE: once for (o0, o1) together, once for o2.
"""
import sys

sys.path.insert(0, "/opt/trn_rl_repo")

import numpy as np
import concourse.bass as bass
import concourse.tile as tile
from concourse import mybir
from concourse.bass_utils import run_bass_kernel_spmd
from contextlib import ExitStack

N_CORES = 8
B, N, D = 128, 2048, 128
BC = B // N_CORES          # batch rows per core
CH = N // 128              # neighbor chunks of 128
AF = mybir.ActivationFunctionType
ALU = mybir.AluOpType
FP32 = mybir.dt.float32
BF16 = mybir.dt.bfloat16
FP8 = mybir.dt.float8e4
USE_FP8 = False            # fp8e4m3 DoubleRow for the o-passes
P8SCALE = 256.0
BIG = 3.0e4                # mask-out offset for the m1 max (fp32-safe)
CLAMP = 60.0               # overflow guard on exp() arguments

_mwctr = [0]


def _split_multiwaits(nc):
    """This walrus build rejects >1 sync-wait per instruction; hoist extras
    onto standalone EventSemaphore instructions on the same engine."""
    for fn in nc.m.functions:
        for bb in fn.blocks:
            new_list = []
            changed = False
            for ins in bb.instructions:
                si = getattr(ins, "sync_info", None)
                on_wait = list(si.on_wait) if si is not None else []
                if len(on_wait) > 1:
                    changed = True
                    for w in on_wait[:-1]:
                        _mwctr[0] += 1
                        ev = mybir.InstEventSemaphore(
                            name=f"I-mwfix-{_mwctr[0]}", ins=[], outs=[])
                        ev.engine = ins.engine
                        ev.debug = ins.debug
                        ev.sync_info = mybir.SyncInfo(on_wait=[w], on_update=[])
                        new_list.append(ev)
                        nc.register_instruction(ev, overwrite=True)
                    si.on_wait = [on_wait[-1]]
                    ins.sync_info = si
                new_list.append(ins)
            if changed:
                live = bb.instructions
                live[:] = new_list


def _build():
    nc = bass.Bass()
    value = nc.dram_tensor("value", [BC, N, D], FP32, kind="ExternalInput")
    mask_t = nc.dram_tensor("mask_t", [128, CH, BC], FP32, kind="ExternalInput")
    e1_t = nc.dram_tensor("e1_t", [D, BC], FP32, kind="ExternalInput")
    w_lhsT = nc.dram_tensor("w_lhsT", [D, D], FP32, kind="ExternalInput")
    b_col = nc.dram_tensor("b_col", [D, 1], FP32, kind="ExternalInput")
    wfu_in = nc.dram_tensor("wfu", [D, 2], FP32, kind="ExternalInput")
    attb_in = nc.dram_tensor("attb", [1, 1], FP32, kind="ExternalInput")
    ident_in = nc.dram_tensor("ident", [128, 128], FP32, kind="ExternalInput")
    y = nc.dram_tensor("y", [BC, D], FP32, kind="ExternalOutput")

    with tile.TileContext(nc) as tc, ExitStack() as ctx:
        P = lambda **kw: ctx.enter_context(tc.tile_pool(**kw))
        sb = P(name="sb", bufs=1)                       # persistent singles
        vt = P(name="vt", bufs=4)                       # transpose staging
        wk = P(name="wk", bufs=3)                       # temporaries
        ps_tr = P(name="ps_tr", bufs=2, space="PSUM")   # transpose batches
        ps_acc = P(name="ps_acc", bufs=2, space="PSUM")  # pass A/B accumulators
        ps_vv = P(name="ps_vv", bufs=2, space="PSUM")    # vs/vu collectors
        ps_sm = P(name="ps_sm", bufs=2, space="PSUM")   # small matmul outs

        # ---- init: small params ----
        w_sb = sb.tile([D, D], FP32, tag="w_sb")
        nc.sync.dma_start(out=w_sb, in_=w_lhsT[:, :])
        bcol_sb = sb.tile([D, 1], FP32, tag="bcol")
        nc.sync.dma_start(out=bcol_sb, in_=b_col[:, :])
        wfu_sb = sb.tile([D, 2], FP32, tag="wfu")
        nc.sync.dma_start(out=wfu_sb, in_=wfu_in[:, :])
        attb_sb = sb.tile([1, 1], FP32, tag="attb")
        nc.sync.dma_start(out=attb_sb, in_=attb_in[:, :])
        identf = sb.tile([128, 128], FP32, tag="identf")
        nc.sync.dma_start(out=identf, in_=ident_in[:, :])
        u0 = sb.tile([D, BC], FP32, tag="u0")
        nc.sync.dma_start(out=u0, in_=e1_t[:, :])

        identb = sb.tile([128, 128], BF16, tag="identb")
        nc.vector.tensor_copy(identb, identf)
        wfu_bf = sb.tile([D, 2], BF16, tag="wfub")
        nc.vector.tensor_copy(wfu_bf, wfu_sb)
        ones_col = sb.tile([128, 1], FP32, tag="onesc")
        nc.vector.memset(ones_col, 1.0)
        ones_row = sb.tile([1, 128], FP32, tag="onesr")
        nc.vector.memset(ones_row, 1.0)
        ln1e5 = sb.tile([1, 1], FP32, tag="ln1e5")
        nc.vector.memset(ln1e5, -11.512925464970229)

        # ---- V loads (bf16 cast) ----
        v_sb = []
        for b in range(BC):
            vtile = sb.tile([128, CH, D], BF16, tag=f"v{b}")
            if b < 2:
                half = value[b].rearrange("(p j) d -> p j d", p=128)
                nc.gpsimd.dma_start(out=vtile[:, 0:CH // 2, :],
                                    in_=half[:, 0:CH // 2, :])
                nc.gpsimd.dma_start(out=vtile[:, CH // 2:, :],
                                    in_=half[:, CH // 2:, :])
            else:
                nc.gpsimd.dma_start(
                    out=vtile,
                    in_=value[b].rearrange("(p j) d -> p j d", p=128))
            v_sb.append(vtile)

        # mask: pre-permuted [128, CH, BC] fp32 -> bf16
        mask_sb = sb.tile([128, CH, BC], BF16, tag="mask")
        nc.gpsimd.dma_start(out=mask_sb, in_=mask_t[:, :, :])

        vsvu = sb.tile([128, CH, BC, 2], FP32, tag="vsvu")
        E = sb.tile([128, CH, BC], BF16, tag="E")
        m1 = sb.tile([1, BC], FP32, tag="m1")
        num01 = sb.tile([128, CH, BC, 2], FP8 if USE_FP8 else BF16,
                        tag="num01")
        num2 = sb.tile([128, CH, BC, 2], FP8 if USE_FP8 else BF16,
                       tag="num2")
        if USE_FP8:
            nc.vector.memset(num2, 0.0)
            v8_sb = []
            for b in range(BC):
                v8t = sb.tile([128, CH, D], FP8, tag=f"v8_{b}")
                v8_sb.append(v8t)
        o01 = sb.tile([128, BC, 2], FP32, tag="o01")
        o2 = sb.tile([128, BC], FP32, tag="o2")
        y_sb = sb.tile([BC, 128], FP32, tag="ysb")

        GB = 16                      # batch rows per pipeline group
        NG = BC // GB

        # ---- helpers (b-group sliced; g0 = first row of group) ----
        def dot_wu(rhs_tile):
            ps = ps_sm.tile([1, GB], FP32, tag="sm")
            nc.tensor.matmul(ps, lhsT=wfu_sb[:, 1:2], rhs=rhs_tile,
                             start=True, stop=True)
            return ps

        def bcast_row(src_1xg):
            ps = ps_sm.tile([128, GB], FP32, tag="sm")
            nc.tensor.matmul(ps, lhsT=ones_row, rhs=src_1xg,
                             start=True, stop=True)
            return ps

        def colsum(red_tile):
            ps = ps_sm.tile([1, GB], FP32, tag="sm")
            nc.tensor.matmul(ps, lhsT=ones_col, rhs=red_tile,
                             start=True, stop=True)
            return ps

        def hop_scalars(c_sb, m1g, tg):
            # max_v = max(relu(m1+c), relu(c)) = relu(c + relu(m1)); m1g holds relu(m1)
            a = wk.tile([1, GB], FP32, tag=f"hs_a{tg}")
            nc.vector.tensor_tensor(out=a, in0=m1g, in1=c_sb, op=ALU.add)
            nc.vector.tensor_scalar(out=a, in0=a, scalar1=0.0, scalar2=CLAMP,
                                    op0=ALU.max, op1=ALU.min)
            corr = wk.tile([1, GB], FP32, tag=f"corr{tg}")
            nc.scalar.activation(out=corr, in_=a, func=AF.Exp,
                                 bias=ln1e5)
            tcl = wk.tile([1, GB], FP32, tag=f"hs_t{tg}")
            nc.vector.tensor_scalar_min(tcl, c_sb, CLAMP)
            texp = wk.tile([1, GB], FP32, tag=f"texp{tg}")
            nc.scalar.activation(out=texp, in_=tcl, func=AF.Exp)
            return bcast_row(texp), corr

        def make_num(t_bc, num_out_view, Eg, maskg, tg):
            tmp = wk.tile([128, CH, GB], BF16, tag=f"numt{tg}")
            nc.vector.tensor_tensor(
                out=tmp, in0=Eg,
                in1=bass.AP(tensor=t_bc.tensor, offset=t_bc.offset,
                            ap=[t_bc.ap[0], [0, CH], t_bc.ap[1]]),
                op=ALU.mult)
            nc.vector.tensor_scalar_max(tmp, tmp, 1.0)
            if not USE_FP8:
                nc.vector.tensor_tensor(
                    out=num_out_view, in0=tmp, in1=maskg, op=ALU.mult)
                return num_out_view
            numbf = wk.tile([128, CH, GB], BF16, tag=f"numb{tg}")
            nc.vector.tensor_tensor(out=numbf, in0=tmp, in1=maskg,
                                    op=ALU.mult)
            return numbf

        def scale_num_fp8(numbf, recip, num_out_view):
            r256 = wk.tile([1, GB], FP32, tag="r256")
            nc.vector.tensor_scalar_mul(r256, recip, P8SCALE)
            rbc = bcast_row(r256)
            nc.vector.tensor_tensor(
                out=num_out_view, in0=numbf,
                in1=bass.AP(tensor=rbc.tensor, offset=rbc.offset,
                            ap=[rbc.ap[0], [0, CH], rbc.ap[1]]),
                op=ALU.mult)

        def denom(num_view, corr, tg):
            red = wk.tile([128, GB], FP32, tag=f"dred{tg}")
            nc.vector.tensor_reduce(
                out=red, in_=num_view.rearrange("p c b -> p b c"),
                axis=mybir.AxisListType.X, op=ALU.add)
            ps = colsum(red)
            d_sb = wk.tile([1, GB], FP32, tag=f"D{tg}")
            nc.vector.tensor_tensor(out=d_sb, in0=ps, in1=corr, op=ALU.add)
            recip = wk.tile([1, GB], FP32, tag=f"recip{tg}")
            nc.vector.reciprocal(recip, d_sb)
            return d_sb, recip

        def weighted_sum_vu(num_view, vu_g, recip, tg):
            nv = wk.tile([128, CH, GB], FP32, tag=f"nv{tg}")
            nc.vector.tensor_tensor(out=nv, in0=num_view, in1=vu_g,
                                    op=ALU.mult)
            red = wk.tile([128, GB], FP32, tag=f"nvred{tg}")
            nc.vector.tensor_reduce(
                out=red, in_=nv.rearrange("p c b -> p b c"),
                axis=mybir.AxisListType.X, op=ALU.add)
            ps = colsum(red)
            out = wk.tile([1, GB], FP32, tag=f"owu{tg}")
            nc.vector.tensor_tensor(out=out, in0=ps, in1=recip, op=ALU.mult)
            return out

        def lin_relu(u_tile, tg):
            ps = ps_sm.tile([D, GB], FP32, tag="sm")
            nc.tensor.matmul(ps, lhsT=w_sb, rhs=u_tile, start=True, stop=True)
            ub = wk.tile([D, GB], FP32, tag=f"ub{tg}")
            nc.scalar.activation(out=ub, in_=ps, func=AF.Relu,
                                 bias=bcol_sb, scale=1.0)
            return ub

        def make_c(base_u, owu, tg):
            ps = dot_wu(base_u)
            c_sb = wk.tile([1, GB], FP32, tag=f"c{tg}")
            nc.vector.tensor_scalar(
                out=c_sb, in0=ps, scalar1=attb_sb, scalar2=None,
                op0=ALU.add)
            if owu is not None:
                nc.vector.tensor_tensor(out=c_sb, in0=c_sb, in1=owu,
                                        op=ALU.add)
            return c_sb

        # ---- phase functions ----
        def vsvu_phase(g0, gn):
            def tr_group(b, cg):
                tr = ps_tr.tile([128, 512], BF16, tag="tr")
                for i in range(4):
                    c = cg * 4 + i
                    nc.tensor.transpose(
                        out=tr[:, i * 128:(i + 1) * 128],
                        in_=v_sb[b][:, c, :],
                        identity=identb)
                return tr

            def wfu_group(b, cg, tr, acc_vv):
                vt4 = vt.tile([128, 512], BF16, tag="vt4")
                if (b + cg) % 2 == 0:
                    nc.vector.tensor_copy(vt4, tr)
                else:
                    nc.scalar.activation(out=vt4, in_=tr, func=AF.Copy)
                for i in range(4):
                    c = cg * 4 + i
                    nc.tensor.matmul(
                        acc_vv[:, c * 2:(c + 1) * 2],
                        lhsT=vt4[:, i * 128:(i + 1) * 128],
                        rhs=wfu_bf,
                        start=True, stop=True)

            for b0 in range(g0, g0 + gn, 2):
                accs = {}
                for b in (b0, b0 + 1):
                    acct = ps_vv.tile([128, 2 * CH], FP32, tag="accv")
                    accs[b] = acct
                for cg in range(CH // 4):
                    tra = tr_group(b0, cg)
                    trb = tr_group(b0 + 1, cg)
                    wfu_group(b0, cg, tra, accs[b0])
                    wfu_group(b0 + 1, cg, trb, accs[b0 + 1])
                for b in (b0, b0 + 1):
                    nc.vector.tensor_copy(
                        vsvu[:, :, b, :],
                        accs[b].rearrange("p (c h) -> p c h", h=2))
                    if USE_FP8:
                        if b % 2 == 0:
                            nc.vector.tensor_copy(v8_sb[b], v_sb[b])
                        else:
                            nc.scalar.activation(out=v8_sb[b], in_=v_sb[b],
                                                 func=AF.Copy)

        def chain01(g0, gn, gi):
            gsl = slice(g0, g0 + gn)
            maskg = mask_sb[:, :, gsl]
            vs_g = vsvu[:, :, gsl, 0]
            vu_g = vsvu[:, :, gsl, 1]
            Eg = E[:, :, gsl]
            nc.scalar.activation(out=Eg, in_=vs_g, func=AF.Exp)
            s1 = wk.tile([128, CH, GB], FP32, tag="s1")
            nc.vector.tensor_tensor(out=s1, in0=vs_g, in1=maskg, op=ALU.mult)
            nc.vector.scalar_tensor_tensor(
                out=s1, in0=maskg, scalar=BIG, in1=s1,
                op0=ALU.mult, op1=ALU.add)
            nc.vector.tensor_scalar_add(s1, s1, -BIG)
            red_m = wk.tile([128, GB], FP32, tag="redm")
            nc.vector.tensor_reduce(
                out=red_m, in_=s1.rearrange("p c b -> p b c"),
                axis=mybir.AxisListType.X, op=ALU.max)
            m1g = m1[:, gsl]
            nc.gpsimd.tensor_reduce(
                out=m1g, in_=red_m, axis=mybir.AxisListType.C, op=ALU.max)
            nc.vector.tensor_scalar_max(m1g, m1g, 0.0)

            u0g = u0[:, gsl]
            c0 = make_c(u0g, None, "0")
            t0bc, corr0 = hop_scalars(c0, m1g, "0")
            nb0 = make_num(t0bc, num01[:, :, gsl, 0], Eg, maskg, "0")
            d0, recip0 = denom(nb0, corr0, f"0_{gi}")
            o0wu = weighted_sum_vu(nb0, vu_g, recip0, f"0_{gi}")
            if USE_FP8:
                scale_num_fp8(nb0, recip0, num01[:, :, gsl, 0])

            ub0 = lin_relu(u0g, f"0_{gi}")
            c1 = make_c(ub0, o0wu, "1")
            t1bc, corr1 = hop_scalars(c1, m1g, "1")
            nb1 = make_num(t1bc, num01[:, :, gsl, 1], Eg, maskg, "1")
            d1, recip1 = denom(nb1, corr1, f"1_{gi}")
            o1wu = weighted_sum_vu(nb1, vu_g, recip1, f"1_{gi}")
            if USE_FP8:
                scale_num_fp8(nb1, recip1, num01[:, :, gsl, 1])
            return dict(ub0=ub0, recip0=recip0, recip1=recip1, o1wu=o1wu,
                        maskg=maskg, Eg=Eg, m1g=m1g, gsl=gsl, gi=gi)

        def passA(g0, gn):
            for b in range(g0, g0 + gn):
                acc = ps_acc.tile([2, 128], FP32, tag="acca")
                if USE_FP8:
                    for cp in range(CH // 2):
                        nc.tensor.matmul(
                            acc,
                            lhsT=num01[:, cp * 2:(cp + 1) * 2, b, :],
                            rhs=v8_sb[b][:, cp * 2:(cp + 1) * 2, :],
                            start=(cp == 0), stop=(cp == CH // 2 - 1),
                            perf_mode=mybir.MatmulPerfMode.DoubleRow)
                else:
                    for c in range(CH):
                        nc.tensor.matmul(
                            acc, lhsT=num01[:, c, b, :], rhs=v_sb[b][:, c, :],
                            start=(c == 0), stop=(c == CH - 1))
                oa_sb = wk.tile([2, 128], FP32, tag="oa")
                nc.scalar.activation(out=oa_sb, in_=acc, func=AF.Copy)
                ps_t = ps_sm.tile([128, 2], FP32, tag="sm")
                nc.tensor.transpose(out=ps_t, in_=oa_sb,
                                    identity=identf[0:2, 0:2])
                nc.vector.tensor_copy(o01[:, b, :], ps_t)

        def chain2(st):
            gsl = st["gsl"]
            u1 = wk.tile([D, GB], FP32, tag="u1")
            if USE_FP8:
                nc.vector.scalar_tensor_tensor(
                    out=u1, in0=o01[:, gsl, 0], scalar=1.0 / P8SCALE,
                    in1=st["ub0"], op0=ALU.mult, op1=ALU.add)
            else:
                r0bc = bcast_row(st["recip0"])
                nc.vector.tensor_tensor(out=u1, in0=o01[:, gsl, 0],
                                        in1=r0bc, op=ALU.mult)
                nc.vector.tensor_tensor(out=u1, in0=u1, in1=st["ub0"],
                                        op=ALU.add)
            ub1 = lin_relu(u1, f"1_{st['gi']}")
            c2 = make_c(ub1, st["o1wu"], "2")
            t2bc, corr2 = hop_scalars(c2, st["m1g"], "2")
            nb2 = make_num(t2bc, num2[:, :, gsl, 0], st["Eg"], st["maskg"],
                           "2")
            d2, recip2 = denom(nb2, corr2, f"2_{st['gi']}")
            if USE_FP8:
                scale_num_fp8(nb2, recip2, num2[:, :, gsl, 0])

            u2 = wk.tile([D, GB], FP32, tag="u2")
            if USE_FP8:
                nc.vector.scalar_tensor_tensor(
                    out=u2, in0=o01[:, gsl, 1], scalar=1.0 / P8SCALE,
                    in1=ub1, op0=ALU.mult, op1=ALU.add)
            else:
                r1bc = bcast_row(st["recip1"])
                nc.vector.tensor_tensor(out=u2, in0=o01[:, gsl, 1],
                                        in1=r1bc, op=ALU.mult)
                nc.vector.tensor_tensor(out=u2, in0=u2, in1=ub1, op=ALU.add)
            ub2 = lin_relu(u2, f"2_{st['gi']}")
            st.update(ub2=ub2, recip2=recip2)

        def passB(g0, gn):
            for b in range(g0, g0 + gn):
                if USE_FP8:
                    acc = ps_acc.tile([2, 128], FP32, tag="acca")
                    for cp in range(CH // 2):
                        nc.tensor.matmul(
                            acc,
                            lhsT=num2[:, cp * 2:(cp + 1) * 2, b, :],
                            rhs=v8_sb[b][:, cp * 2:(cp + 1) * 2, :],
                            start=(cp == 0), stop=(cp == CH // 2 - 1),
                            perf_mode=mybir.MatmulPerfMode.DoubleRow)
                    ob_sb = wk.tile([2, 128], FP32, tag="ob")
                    if b % 2 == 0:
                        nc.vector.tensor_copy(ob_sb, acc)
                    else:
                        nc.scalar.activation(out=ob_sb, in_=acc,
                                             func=AF.Copy)
                    ps_t = ps_sm.tile([128, 2], FP32, tag="sm")
                    nc.tensor.transpose(out=ps_t, in_=ob_sb[0:2, :],
                                        identity=identf[0:2, 0:2])
                    nc.vector.tensor_copy(o2[:, b:b + 1], ps_t[:, 0:1])
                    continue
                acc = ps_acc.tile([1, 128], FP32, tag="acca")
                for c in range(CH):
                    nc.tensor.matmul(
                        acc, lhsT=num2[:, c, b, 0:1], rhs=v_sb[b][:, c, :],
                        start=(c == 0), stop=(c == CH - 1))
                ob_sb = wk.tile([1, 128], FP32, tag="ob")
                nc.scalar.activation(out=ob_sb, in_=acc, func=AF.Copy)
                ps_t = ps_sm.tile([128, 1], FP32, tag="sm")
                nc.tensor.transpose(out=ps_t, in_=ob_sb,
                                    identity=identf[0:1, 0:1])
                nc.vector.tensor_copy(o2[:, b:b + 1], ps_t)

        def finish(st, g0, gn):
            gsl = st["gsl"]
            u3 = wk.tile([D, GB], FP32, tag="u3")
            if USE_FP8:
                nc.vector.scalar_tensor_tensor(
                    out=u3, in0=o2[:, gsl], scalar=1.0 / P8SCALE,
                    in1=st["ub2"], op0=ALU.mult, op1=ALU.add)
            else:
                r2bc = bcast_row(st["recip2"])
                nc.vector.tensor_tensor(out=u3, in0=o2[:, gsl], in1=r2bc,
                                        op=ALU.mult)
                nc.vector.tensor_tensor(out=u3, in0=u3, in1=st["ub2"],
                                        op=ALU.add)
            ps_y = ps_sm.tile([GB, 128], FP32, tag="sm")
            nc.tensor.transpose(out=ps_y, in_=u3, identity=identf)
            yg = wk.tile([GB, 128], FP32, tag="yg")
            nc.vector.tensor_copy(yg, ps_y)
            nc.sync.dma_start(out=y[g0:g0 + gn, :], in_=yg)

        # ---- grouped software pipeline ----
        sts = []
        for g in range(NG):
            vsvu_phase(g * GB, GB)
            sts.append(chain01(g * GB, GB, g))
            passA(g * GB, GB)
        for g in range(NG):
            chain2(sts[g])
            passB(g * GB, GB)
            finish(sts[g], g * GB, GB)

    _split_multiwaits(nc)
    return nc


_nc_cache = None


def _get_nc():
    global _nc_cache
    if _nc_cache is None:
        _nc_cache = _build()
    return _nc_cache


def kernel(**inputs):
    e1 = np.asarray(inputs["e1_embeded"], dtype=np.float32)
    value = np.asarray(inputs["nei_embeded_value"], dtype=np.float32)
    mask = np.asarray(inputs["nei_mask"], dtype=np.float32)
    linfc_w = np.asarray(inputs["linfc_w"], dtype=np.float32)
    linfc_b = np.asarray(inputs["linfc_b"], dtype=np.float32)
    attfc_w = np.asarray(inputs["attfc_w"], dtype=np.float32)
    attfc_b = np.asarray(inputs["attfc_b"], dtype=np.float32)

    w_lhsT = np.ascontiguousarray(linfc_w.T)
    b_col = np.ascontiguousarray(linfc_b.reshape(D, 1))
    wfu = np.ascontiguousarray(
        np.stack([attfc_w[0, :D], attfc_w[0, D:]], axis=1))
    attb = np.asarray(attfc_b, dtype=np.float32).reshape(1, 1)
    ident = np.eye(128, dtype=np.float32)

    in_maps = []
    for core in range(N_CORES):
        b0 = core * BC
        in_maps.append({
            "value": np.ascontiguousarray(value[b0:b0 + BC]),
            "mask_t": np.ascontiguousarray(np.transpose(
                mask[b0:b0 + BC].reshape(BC, 128, CH), (1, 2, 0))),
            "e1_t": np.ascontiguousarray(e1[b0:b0 + BC].T),
            "w_lhsT": w_lhsT,
            "b_col": b_col,
            "wfu": wfu,
            "attb": attb,
            "ident": ident,
        })

    nc = _get_nc()
    res = run_bass_kernel_spmd(nc, in_maps, list(range(N_CORES)))
    out = np.concatenate([res.results[i]["y"] for i in range(N_CORES)], axis=0)
    return out.astype(np.float32)



# revision 2
# speedup vs baseline: 1.4085x; 1.4085x over previous
"""DMN encoder (3-hop masked-attention message passing) on 8 trn2 cores.

Sharding: pure data-parallel over the batch dim (16 rows/core).

v3 design vs baseline:
  - host pre-casts V to bf16 and pre-transposes it, so the device gets
    BOTH layouts (vn: neighbors-on-partitions, vt: d-on-partitions) via
    plain HWDGE DMA - no SWDGE casts, no PE transposes.
  - vs/vu per-neighbor dots: lhsT = vt chunk (stationary), rhs = wfu
    -> out [128(p), 2] lands directly in the layout the softmax needs.
  - o-passes: lhsT = vn chunk (stationary), rhs = num columns
    -> out [128(d), hops] accumulated across chunks in PSUM, no output
    transpose; one PSUM->SBUF copy per row-group.
"""
import sys

sys.path.insert(0, "/opt/trn_rl_repo")

import numpy as np
import ml_dtypes
import concourse.bass as bass
import concourse.tile as tile
from concourse import mybir
from concourse.bass_utils import run_bass_kernel_spmd
from contextlib import ExitStack

N_CORES = 8
B, N, D = 128, 2048, 128
BC = B // N_CORES          # batch rows per core
CH = N // 128              # neighbor chunks of 128
GB = 8                     # batch rows per pipeline group
NG = BC // GB
AF = mybir.ActivationFunctionType
ALU = mybir.AluOpType
FP32 = mybir.dt.float32
BF16 = mybir.dt.bfloat16
BIG = 3.0e4                # mask-out offset for the m1 max (fp32-safe)
CLAMP = 60.0               # overflow guard on exp() arguments

_mwctr = [0]


def _split_multiwaits(nc):
    """This walrus build rejects >1 sync-wait per instruction; hoist extras
    onto standalone EventSemaphore instructions on the same engine."""
    for fn in nc.m.functions:
        for bb in fn.blocks:
            new_list = []
            changed = False
            for ins in bb.instructions:
                si = getattr(ins, "sync_info", None)
                on_wait = list(si.on_wait) if si is not None else []
                if len(on_wait) > 1:
                    changed = True
                    for w in on_wait[:-1]:
                        _mwctr[0] += 1
                        ev = mybir.InstEventSemaphore(
                            name=f"I-mwfix-{_mwctr[0]}", ins=[], outs=[])
                        ev.engine = ins.engine
                        ev.debug = ins.debug
                        ev.sync_info = mybir.SyncInfo(on_wait=[w], on_update=[])
                        new_list.append(ev)
                        nc.register_instruction(ev, overwrite=True)
                    si.on_wait = [on_wait[-1]]
                    ins.sync_info = si
                new_list.append(ins)
            if changed:
                live = bb.instructions
                live[:] = new_list


def _build():
    nc = bass.Bass()
    vn_in = nc.dram_tensor("vn", [128, BC, CH, D], BF16, kind="ExternalInput")
    vt_in = nc.dram_tensor("vt", [128, BC, CH, 128], BF16,
                           kind="ExternalInput")
    mask_in = nc.dram_tensor("mask_t", [128, CH, BC], BF16,
                             kind="ExternalInput")
    e1_t = nc.dram_tensor("e1_t", [D, BC], FP32, kind="ExternalInput")
    w_lhsT = nc.dram_tensor("w_lhsT", [D, D], FP32, kind="ExternalInput")
    b_col = nc.dram_tensor("b_col", [D, 1], FP32, kind="ExternalInput")
    wfu_in = nc.dram_tensor("wfu", [D, 2], FP32, kind="ExternalInput")
    attb_in = nc.dram_tensor("attb", [1, 1], FP32, kind="ExternalInput")
    ident_in = nc.dram_tensor("ident", [128, 128], FP32, kind="ExternalInput")
    y = nc.dram_tensor("y", [BC, D], FP32, kind="ExternalOutput")

    with tile.TileContext(nc) as tc, ExitStack() as ctx:
        P = lambda **kw: ctx.enter_context(tc.tile_pool(**kw))
        sb = P(name="sb", bufs=1)                       # persistent singles
        wk = P(name="wk", bufs=3)                       # temporaries
        ps_vv = P(name="ps_vv", bufs=2, space="PSUM")   # vs/vu collectors
        ps_oA = P(name="ps_oA", bufs=2, space="PSUM")   # passA accumulators
        ps_oB = P(name="ps_oB", bufs=2, space="PSUM")   # passB accumulators
        ps_sm = P(name="ps_sm", bufs=2, space="PSUM")   # small matmul outs

        # ---- init: small params ----
        w_sb = sb.tile([D, D], FP32, tag="w_sb")
        nc.sync.dma_start(out=w_sb, in_=w_lhsT[:, :])
        bcol_sb = sb.tile([D, 1], FP32, tag="bcol")
        nc.sync.dma_start(out=bcol_sb, in_=b_col[:, :])
        wfu_sb = sb.tile([D, 2], FP32, tag="wfu")
        nc.sync.dma_start(out=wfu_sb, in_=wfu_in[:, :])
        attb_sb = sb.tile([1, 1], FP32, tag="attb")
        nc.sync.dma_start(out=attb_sb, in_=attb_in[:, :])
        identf = sb.tile([128, 128], FP32, tag="identf")
        nc.sync.dma_start(out=identf, in_=ident_in[:, :])
        u0 = sb.tile([D, BC], FP32, tag="u0")
        nc.sync.dma_start(out=u0, in_=e1_t[:, :])
        mask_sb = sb.tile([128, CH, BC], BF16, tag="mask")
        nc.sync.dma_start(out=mask_sb, in_=mask_in[:, :, :])

        wfu_bf = sb.tile([D, 2], BF16, tag="wfub")
        nc.vector.tensor_copy(wfu_bf, wfu_sb)
        ones_col = sb.tile([128, 1], FP32, tag="onesc")
        nc.vector.memset(ones_col, 1.0)
        ones_row = sb.tile([1, 128], FP32, tag="onesr")
        nc.vector.memset(ones_row, 1.0)
        ln1e5 = sb.tile([1, 1], FP32, tag="ln1e5")
        nc.vector.memset(ln1e5, -11.512925464970229)

        # ---- big V loads, group-sliced for pipelining ----
        vt_sb = sb.tile([128, BC, CH, 128], BF16, tag="vt")
        vn_sb = sb.tile([128, BC, CH, D], BF16, tag="vn")
        for g in range(NG):
            gsl = slice(g * GB, (g + 1) * GB)
            nc.sync.dma_start(out=vt_sb[:, gsl, :, :], in_=vt_in[:, gsl, :, :])
            nc.sync.dma_start(out=vn_sb[:, gsl, :, :], in_=vn_in[:, gsl, :, :])

        vsvu = sb.tile([128, CH, BC, 2], FP32, tag="vsvu")
        E = sb.tile([128, CH, BC], BF16, tag="E")
        m1 = sb.tile([1, BC], FP32, tag="m1")
        num01 = sb.tile([128, CH, BC, 2], BF16, tag="num01")
        num2 = sb.tile([128, CH, BC, 1], BF16, tag="num2")
        o01 = sb.tile([128, BC, 2], FP32, tag="o01")
        o2 = sb.tile([128, BC], FP32, tag="o2")

        # ---- helpers (b-group sliced) ----
        def dot_wu(rhs_tile):
            ps = ps_sm.tile([1, GB], FP32, tag="sm")
            nc.tensor.matmul(ps, lhsT=wfu_sb[:, 1:2], rhs=rhs_tile,
                             start=True, stop=True)
            return ps

        def bcast_row(src_1xg):
            ps = ps_sm.tile([128, GB], FP32, tag="sm")
            nc.tensor.matmul(ps, lhsT=ones_row, rhs=src_1xg,
                             start=True, stop=True)
            return ps

        def colsum(red_tile):
            ps = ps_sm.tile([1, GB], FP32, tag="sm")
            nc.tensor.matmul(ps, lhsT=ones_col, rhs=red_tile,
                             start=True, stop=True)
            return ps

        def hop_scalars(c_sb, m1g, tg):
            # max_v = max(relu(m1+c), relu(c)) = relu(c + relu(m1)); m1g holds relu(m1)
            a = wk.tile([1, GB], FP32, tag=f"hs_a{tg}")
            nc.vector.tensor_tensor(out=a, in0=m1g, in1=c_sb, op=ALU.add)
            nc.vector.tensor_scalar(out=a, in0=a, scalar1=0.0, scalar2=CLAMP,
                                    op0=ALU.max, op1=ALU.min)
            corr = wk.tile([1, GB], FP32, tag=f"corr{tg}")
            nc.scalar.activation(out=corr, in_=a, func=AF.Exp,
                                 bias=ln1e5)
            tcl = wk.tile([1, GB], FP32, tag=f"hs_t{tg}")
            nc.vector.tensor_scalar_min(tcl, c_sb, CLAMP)
            texp = wk.tile([1, GB], FP32, tag=f"texp{tg}")
            nc.scalar.activation(out=texp, in_=tcl, func=AF.Exp)
            return bcast_row(texp), corr

        def make_num(t_bc, num_out_view, Eg, maskg, tg):
            tmp = wk.tile([128, CH, GB], BF16, tag=f"numt{tg}")
            nc.vector.tensor_tensor(
                out=tmp, in0=Eg,
                in1=bass.AP(tensor=t_bc.tensor, offset=t_bc.offset,
                            ap=[t_bc.ap[0], [0, CH], t_bc.ap[1]]),
                op=ALU.mult)
            nc.vector.tensor_scalar_max(tmp, tmp, 1.0)
            nc.vector.tensor_tensor(
                out=num_out_view, in0=tmp, in1=maskg, op=ALU.mult)
            return num_out_view

        def denom(num_view, corr, tg):
            red = wk.tile([128, GB], FP32, tag=f"dred{tg}")
            nc.vector.tensor_reduce(
                out=red, in_=num_view.rearrange("p c b -> p b c"),
                axis=mybir.AxisListType.X, op=ALU.add)
            ps = colsum(red)
            d_sb = wk.tile([1, GB], FP32, tag=f"D{tg}")
            nc.vector.tensor_tensor(out=d_sb, in0=ps, in1=corr, op=ALU.add)
            recip = wk.tile([1, GB], FP32, tag=f"recip{tg}")
            nc.vector.reciprocal(recip, d_sb)
            return d_sb, recip

        def weighted_sum_vu(num_view, vu_g, recip, tg):
            nv = wk.tile([128, CH, GB], FP32, tag=f"nv{tg}")
            nc.vector.tensor_tensor(out=nv, in0=num_view, in1=vu_g,
                                    op=ALU.mult)
            red = wk.tile([128, GB], FP32, tag=f"nvred{tg}")
            nc.vector.tensor_reduce(
                out=red, in_=nv.rearrange("p c b -> p b c"),
                axis=mybir.AxisListType.X, op=ALU.add)
            ps = colsum(red)
            out = wk.tile([1, GB], FP32, tag=f"owu{tg}")
            nc.vector.tensor_tensor(out=out, in0=ps, in1=recip, op=ALU.mult)
            return out

        def lin_relu(u_tile, tg):
            ps = ps_sm.tile([D, GB], FP32, tag="sm")
            nc.tensor.matmul(ps, lhsT=w_sb, rhs=u_tile, start=True, stop=True)
            ub = wk.tile([D, GB], FP32, tag=f"ub{tg}")
            nc.scalar.activation(out=ub, in_=ps, func=AF.Relu,
                                 bias=bcol_sb, scale=1.0)
            return ub

        def make_c(base_u, owu, tg):
            ps = dot_wu(base_u)
            c_sb = wk.tile([1, GB], FP32, tag=f"c{tg}")
            nc.vector.tensor_scalar(
                out=c_sb, in0=ps, scalar1=attb_sb, scalar2=None,
                op0=ALU.add)
            if owu is not None:
                nc.vector.tensor_tensor(out=c_sb, in0=c_sb, in1=owu,
                                        op=ALU.add)
            return c_sb

        # ---- phase functions ----
        def vsvu_phase(g0, gn):
            for b in range(g0, g0 + gn):
                acc = ps_vv.tile([128, 2 * CH], FP32, tag="accv")
                for c in range(CH):
                    nc.tensor.matmul(
                        acc[:, c * 2:(c + 1) * 2],
                        lhsT=vt_sb[:, b, c, :], rhs=wfu_bf,
                        start=True, stop=True)
                if b % 2 == 0:
                    nc.vector.tensor_copy(
                        vsvu[:, :, b, :],
                        acc.rearrange("p (c h) -> p c h", h=2))
                else:
                    nc.scalar.activation(
                        out=vsvu[:, :, b, :],
                        in_=acc.rearrange("p (c h) -> p c h", h=2),
                        func=AF.Copy)

        def chain01(g0, gn, gi):
            gsl = slice(g0, g0 + gn)
            maskg = mask_sb[:, :, gsl]
            vs_g = vsvu[:, :, gsl, 0]
            vu_g = vsvu[:, :, gsl, 1]
            Eg = E[:, :, gsl]
            nc.scalar.activation(out=Eg, in_=vs_g, func=AF.Exp)
            s1 = wk.tile([128, CH, GB], FP32, tag="s1")
            nc.vector.tensor_tensor(out=s1, in0=vs_g, in1=maskg, op=ALU.mult)
            nc.vector.scalar_tensor_tensor(
                out=s1, in0=maskg, scalar=BIG, in1=s1,
                op0=ALU.mult, op1=ALU.add)
            nc.vector.tensor_scalar_add(s1, s1, -BIG)
            red_m = wk.tile([128, GB], FP32, tag="redm")
            nc.vector.tensor_reduce(
                out=red_m, in_=s1.rearrange("p c b -> p b c"),
                axis=mybir.AxisListType.X, op=ALU.max)
            m1g = m1[:, gsl]
            nc.gpsimd.tensor_reduce(
                out=m1g, in_=red_m, axis=mybir.AxisListType.C, op=ALU.max)
            nc.vector.tensor_scalar_max(m1g, m1g, 0.0)

            u0g = u0[:, gsl]
            c0 = make_c(u0g, None, "0")
            t0bc, corr0 = hop_scalars(c0, m1g, "0")
            nb0 = make_num(t0bc, num01[:, :, gsl, 0], Eg, maskg, "0")
            d0, recip0 = denom(nb0, corr0, f"0_{gi}")
            o0wu = weighted_sum_vu(nb0, vu_g, recip0, f"0_{gi}")

            ub0 = lin_relu(u0g, f"0_{gi}")
            c1 = make_c(ub0, o0wu, "1")
            t1bc, corr1 = hop_scalars(c1, m1g, "1")
            nb1 = make_num(t1bc, num01[:, :, gsl, 1], Eg, maskg, "1")
            d1, recip1 = denom(nb1, corr1, f"1_{gi}")
            o1wu = weighted_sum_vu(nb1, vu_g, recip1, f"1_{gi}")
            return dict(ub0=ub0, recip0=recip0, recip1=recip1, o1wu=o1wu,
                        maskg=maskg, Eg=Eg, m1g=m1g, gsl=gsl, gi=gi)

        def passA(g0, gn):
            acc = ps_oA.tile([128, 2 * GB], FP32, tag="acca")
            for b in range(g0, g0 + gn):
                bb = b - g0
                for c in range(CH):
                    nc.tensor.matmul(
                        acc[:, bb * 2:(bb + 1) * 2],
                        lhsT=vn_sb[:, b, c, :], rhs=num01[:, c, b, :],
                        start=(c == 0), stop=(c == CH - 1))
            nc.vector.tensor_copy(
                o01[:, g0:g0 + gn, :],
                acc.rearrange("p (b h) -> p b h", h=2))

        def chain2(st):
            gsl = st["gsl"]
            u1 = wk.tile([D, GB], FP32, tag="u1")
            r0bc = bcast_row(st["recip0"])
            nc.vector.tensor_tensor(out=u1, in0=o01[:, gsl, 0],
                                    in1=r0bc, op=ALU.mult)
            nc.vector.tensor_tensor(out=u1, in0=u1, in1=st["ub0"],
                                    op=ALU.add)
            ub1 = lin_relu(u1, f"1_{st['gi']}")
            c2 = make_c(ub1, st["o1wu"], "2")
            t2bc, corr2 = hop_scalars(c2, st["m1g"], "2")
            nb2 = make_num(t2bc, num2[:, :, gsl, 0], st["Eg"], st["maskg"],
                           "2")
            d2, recip2 = denom(nb2, corr2, f"2_{st['gi']}")

            u2 = wk.tile([D, GB], FP32, tag="u2")
            r1bc = bcast_row(st["recip1"])
            nc.vector.tensor_tensor(out=u2, in0=o01[:, gsl, 1],
                                    in1=r1bc, op=ALU.mult)
            nc.vector.tensor_tensor(out=u2, in0=u2, in1=ub1, op=ALU.add)
            ub2 = lin_relu(u2, f"2_{st['gi']}")
            st.update(ub2=ub2, recip2=recip2)

        def passB(g0, gn):
            acc = ps_oB.tile([128, GB], FP32, tag="accb")
            for b in range(g0, g0 + gn):
                bb = b - g0
                for c in range(CH):
                    nc.tensor.matmul(
                        acc[:, bb:bb + 1],
                        lhsT=vn_sb[:, b, c, :], rhs=num2[:, c, b, :],
                        start=(c == 0), stop=(c == CH - 1))
            nc.scalar.activation(out=o2[:, g0:g0 + gn], in_=acc,
                                 func=AF.Copy)

        def finish(st, g0, gn):
            gsl = st["gsl"]
            u3 = wk.tile([D, GB], FP32, tag="u3")
            r2bc = bcast_row(st["recip2"])
            nc.vector.tensor_tensor(out=u3, in0=o2[:, gsl], in1=r2bc,
                                    op=ALU.mult)
            nc.vector.tensor_tensor(out=u3, in0=u3, in1=st["ub2"],
                                    op=ALU.add)
            ps_y = ps_sm.tile([GB, 128], FP32, tag="sm")
            nc.tensor.transpose(out=ps_y, in_=u3, identity=identf)
            yg = wk.tile([GB, 128], FP32, tag="yg")
            nc.vector.tensor_copy(yg, ps_y)
            nc.sync.dma_start(out=y[g0:g0 + gn, :], in_=yg)

        # ---- grouped software pipeline ----
        sts = []
        for g in range(NG):
            vsvu_phase(g * GB, GB)
            sts.append(chain01(g * GB, GB, g))
            passA(g * GB, GB)
        for g in range(NG):
            chain2(sts[g])
            passB(g * GB, GB)
            finish(sts[g], g * GB, GB)

    _split_multiwaits(nc)
    return nc


_nc_cache = None


def _get_nc():
    global _nc_cache
    if _nc_cache is None:
        _nc_cache = _build()
    return _nc_cache


def make_in_maps(inputs):
    e1 = np.asarray(inputs["e1_embeded"], dtype=np.float32)
    value = np.asarray(inputs["nei_embeded_value"], dtype=np.float32)
    mask = np.asarray(inputs["nei_mask"], dtype=np.float32)
    linfc_w = np.asarray(inputs["linfc_w"], dtype=np.float32)
    linfc_b = np.asarray(inputs["linfc_b"], dtype=np.float32)
    attfc_w = np.asarray(inputs["attfc_w"], dtype=np.float32)
    attfc_b = np.asarray(inputs["attfc_b"], dtype=np.float32)

    bf16 = ml_dtypes.bfloat16
    w_lhsT = np.ascontiguousarray(linfc_w.T)
    b_col = np.ascontiguousarray(linfc_b.reshape(D, 1))
    wfu = np.ascontiguousarray(
        np.stack([attfc_w[0, :D], attfc_w[0, D:]], axis=1))
    attb = np.asarray(attfc_b, dtype=np.float32).reshape(1, 1)
    ident = np.eye(128, dtype=np.float32)

    in_maps = []
    for core in range(N_CORES):
        b0 = core * BC
        r = value[b0:b0 + BC].reshape(BC, 128, CH, D)
        in_maps.append({
            "vn": r.transpose(1, 0, 2, 3).astype(bf16),
            "vt": r.transpose(3, 0, 2, 1).astype(bf16),
            "mask_t": mask[b0:b0 + BC].reshape(BC, 128, CH)
                      .transpose(1, 2, 0).astype(bf16),
            "e1_t": np.ascontiguousarray(e1[b0:b0 + BC].T),
            "w_lhsT": w_lhsT,
            "b_col": b_col,
            "wfu": wfu,
            "attb": attb,
            "ident": ident,
        })
    return in_maps


def kernel(**inputs):
    in_maps = make_in_maps(inputs)
    nc = _get_nc()
    res = run_bass_kernel_spmd(nc, in_maps, list(range(N_CORES)))
    out = np.concatenate([res.results[i]["y"] for i in range(N_CORES)], axis=0)
    return out.astype(np.float32)
